# revision 8
# baseline (speedup 1.0000x reference)
"""Multi-head attention (B=4, S=2048, D=1024, H=16) on 8 TRN2 NeuronCores.

Sharding: core c <- batch c//2, heads 8*(c%2) .. 8*(c%2)+8 (Megatron-style:
Wq/Wk/Wv column-parallel, Wo row-parallel). No collectives: the two partial
outputs per batch are summed on the host (plus the bo bias).

Per-core kernel strategy:
  - All matmul operands are fp16 (hosts converts inputs): full PE rate,
    4x cheaper LDWEIGHTS via fast-weight-load, half the DMA bytes.
  - q^T, k^T computed directly in [head_dim, seq] layout (out = W^T.T @ X^T),
    v computed in natural [seq, head_dim] layout with a ones column appended.
  - Scores computed transposed: ST[s_k, s_q] = k . q, so softmax exp is pure
    elementwise (no max subtraction needed: scores ~ N(0,1) after 1/8 scale)
    and no on-chip transposes are needed anywhere.
  - ctx^T[c, s_q] accumulated as v_aug^T @ exp(ST); the ones column yields the
    softmax denominator l[s_q] as psum row 64 for free.
  - Normalization by 1/l (reciprocal_approx_fast) before the output proj.
"""
import sys

sys.path.insert(0, "/opt/trn_rl_repo")
import numpy as np

import concourse.bass as bass
import concourse.bacc as bacc
import concourse.mybir as mybir
import concourse.tile as tile
from concourse.bass_utils import run_bass_kernel_spmd

f32 = mybir.dt.float32
f16 = mybir.dt.float16
EXP = mybir.ActivationFunctionType.Exp

S = 2048          # sequence length
D = 1024          # model dim
HC = 8            # heads per core
DK = 64           # head dim
JC = HC * DK      # per-core projection width (512)
SCALE = 0.125     # 1/sqrt(DK)
N_CORES = 8


def _stage1(nc, tc, work, io, sb):
    """QKV projections -> qT_sb, kT_sb, v_sb (with ones column)."""
    nc.vector.memset(sb.v_sb[:, :, :, DK], 1.0)
    with (
        tc.tile_pool(name="x1", bufs=2) as xp,
        tc.tile_pool(name="ps1", bufs=3, space="PSUM") as pp,
        tc.tile_pool(name="ps1v", bufs=2, space="PSUM") as ppv,
    ):
        # q/k projections: w block [128(kt),128(j)] stationary, xT moving.
        # psum [128, 1024] per (jt, chunk) accumulated over kt (LDW:MM=1:2).
        # Order q -> v -> k so the v bias-add DVE tail drains during k-proj
        # instead of stalling the stage-2 psum pool open.
        def qk_proj(x_dram, w_sb, o_sb, b_sb):
            for sc in range(2):  # s chunks of 1024
                xq = xp.tile([128, 8, 1024], f16, tag="x")
                nc.sync.dma_start(
                    xq[:],
                    x_dram[:, sc * 1024:(sc + 1) * 1024].rearrange(
                        "(kt p) s -> p kt s", p=128
                    ),
                )
                for jt in range(4):
                    ps = pp.tile([128, 1024], f32, tag="proj")
                    for kt in range(8):
                        w = w_sb[:, kt, jt * 128:(jt + 1) * 128]
                        nc.tensor.matmul(
                            ps[:, 0:512], w, xq[:, kt, 0:512],
                            start=(kt == 0), stop=(kt == 7),
                        )
                        nc.tensor.matmul(
                            ps[:, 512:1024], w, xq[:, kt, 512:1024],
                            start=(kt == 0), stop=(kt == 7),
                        )
                    nc.vector.tensor_scalar_add(
                        o_sb[:, jt, sc * 1024:(sc + 1) * 1024],
                        ps[:],
                        b_sb[:, jt:jt + 1],
                    )

        def v_proj():
            # xT chunk stationary, wv moving; out [s, j] + bias.
            for st in range(16):
                xv = xp.tile([128, 8, 128], f16, tag="xv")
                nc.sync.dma_start(
                    xv[:],
                    io.vt[:, st * 128:(st + 1) * 128].rearrange(
                        "(kt p) s -> p kt s", p=128
                    ),
                )
                ps = ppv.tile([128, JC], f32, tag="projv")
                for kt in range(8):
                    nc.tensor.matmul(
                        ps[:],
                        xv[:, kt, :],
                        sb.wv_sb[:, kt, :],
                        start=(kt == 0), stop=(kt == 7),
                    )
                nc.vector.tensor_add(
                    sb.v_sb[:, st, :, 0:DK],
                    ps[:].rearrange("p (h c) -> p h c", h=HC),
                    sb.bvb_sb[:].rearrange("p (h c) -> p h c", h=HC),
                )

        qk_proj(io.qt, sb.wq_sb, sb.qT_sb, sb.bq_sb)
        v_proj()
        qk_proj(io.kt, sb.wk_sb, sb.kT_sb, sb.bk_sb)


def _stage2(nc, tc, work, io, sb):
    """Attention: scores^T -> exp -> ctx^T (+denominator) -> normalize.

    Head-serial, s_q blocked by 1024. Each exp covers [128, 1024].
    Scores/exp run one k-iteration ahead of ctx.
    """
    with (
        tc.tile_pool(name="ps2st", bufs=2, space="PSUM") as pp_st,
        tc.tile_pool(name="ps2ctx", bufs=2, space="PSUM") as pp_ctx,
        tc.tile_pool(name="att", bufs=4) as att,
        tc.tile_pool(name="att2", bufs=2) as att2,
    ):
        ctxs = {}

        def emit_ctx(h, sqb, k, pt):
            c0, c1 = ctxs[(h, sqb)]
            vt = sb.v_sb[:, k, h, :]
            nc.tensor.matmul(c0[:], vt, pt[:, 0:512], start=(k == 0), stop=(k == 15))
            nc.tensor.matmul(c1[:], vt, pt[:, 512:1024], start=(k == 0), stop=(k == 15))
            if k == 15:
                jt = h // 2
                pbase = 64 * (h % 2)
                for ci, ctx in enumerate((c0, c1)):
                    sq = sqb * 2 + ci
                    # l row -> SBUF, then cheap approx reciprocal (the
                    # full-rate DVE reciprocal is ~3.3us and was gating the
                    # next block's ctx matmuls).
                    lc = att2.tile([1, 512], f32, tag=f"l{ci}", name=f"l_{h}_{sq}")
                    nc.vector.tensor_copy(lc[:], ctx[DK:DK + 1, :])
                    r = att2.tile([1, 512], f32, tag=f"r{ci}", name=f"r_{h}_{sq}")
                    nc.vector.reciprocal_approx_fast(r[:], lc[:])
                    rb = att2.tile([64, 512], f32, tag=f"rb{ci}", name=f"rb_{h}_{sq}")
                    nc.gpsimd.partition_broadcast(rb[:], r[:])
                    nc.vector.tensor_mul(
                        sb.ctxn_sb[pbase:pbase + 64, jt, sq * 512:(sq + 1) * 512],
                        ctx[0:DK, :], rb[:],
                    )
                del ctxs[(h, sqb)]

        pend = None
        for h in range(8):
            jt = h // 2
            pbase = 64 * (h % 2)
            for sqb in range(2):      # s_q blocks of 1024
                ctxs[(h, sqb)] = (
                    pp_ctx.tile([DK + 1, 512], f32, tag="ctx0", name=f"ctx0_{h}_{sqb}"),
                    pp_ctx.tile([DK + 1, 512], f32, tag="ctx1", name=f"ctx1_{h}_{sqb}"),
                )
                for k in range(16):   # s_k tiles of 128
                    st = pp_st.tile([128, 1024], f32, tag="st")
                    lhs = sb.kT_sb[pbase:pbase + 64, jt, k * 128:(k + 1) * 128]
                    nc.tensor.matmul(
                        st[:, 0:512], lhs,
                        sb.qT_sb[pbase:pbase + 64, jt,
                                 sqb * 1024:sqb * 1024 + 512],
                        start=True, stop=True,
                    )
                    nc.tensor.matmul(
                        st[:, 512:1024], lhs,
                        sb.qT_sb[pbase:pbase + 64, jt,
                                 sqb * 1024 + 512:sqb * 1024 + 1024],
                        start=True, stop=True,
                    )
                    pt = att.tile([128, 1024], f16, tag="pt")
                    nc.scalar.activation(pt[:], st[:], EXP, scale=SCALE)
                    if pend is not None:
                        emit_ctx(*pend)
                    pend = (h, sqb, k, pt)
        emit_ctx(*pend)


def _stage3(nc, tc, work, io, sb):
    """Output projection: out[s, :] = ctxn^T.T @ WoT."""
    with tc.tile_pool(name="ps3", bufs=2, space="PSUM") as pp3:
        for sq2 in range(16):
            for n in range(2):
                ps = pp3.tile([128, 512], f32, tag="o")
                for p in range(4):
                    nc.tensor.matmul(
                        ps[:],
                        sb.ctxn_sb[:, p, sq2 * 128:(sq2 + 1) * 128],
                        sb.wot_sb[:, p, n * 512:(n + 1) * 512],
                        start=(p == 0), stop=(p == 3),
                    )
                ob = work.tile([128, 512], f32, tag="ob")
                nc.vector.tensor_copy(ob[:], ps[:])
                nc.sync.dma_start(
                    io.out[sq2 * 128:(sq2 + 1) * 128, n * 512:(n + 1) * 512],
                    ob[:],
                )


class _NS:
    pass


def build_nc(repeats=1, stages=(1, 2, 3)):
    nc = bacc.Bacc(None, target_bir_lowering=False, debug=False)

    io = _NS()
    io.qt = nc.dram_tensor("qt", [D, S], f16, kind="ExternalInput")
    io.kt = nc.dram_tensor("kt", [D, S], f16, kind="ExternalInput")
    io.vt = nc.dram_tensor("vt", [D, S], f16, kind="ExternalInput")
    io.wqt = nc.dram_tensor("wqt", [D, JC], f16, kind="ExternalInput")
    io.wkt = nc.dram_tensor("wkt", [D, JC], f16, kind="ExternalInput")
    io.wvt = nc.dram_tensor("wvt", [D, JC], f16, kind="ExternalInput")
    io.wot = nc.dram_tensor("wot", [JC, D], f16, kind="ExternalInput")
    io.bq = nc.dram_tensor("bq", [128, 4], f32, kind="ExternalInput")
    io.bk = nc.dram_tensor("bk", [128, 4], f32, kind="ExternalInput")
    io.bvb = nc.dram_tensor("bvb", [128, JC], f32, kind="ExternalInput")
    io.out = nc.dram_tensor("out", [S, D], f32, kind="ExternalOutput")

    with tile.TileContext(nc) as tc:
        for _rep in range(repeats):
            with (
                tc.tile_pool(name="big", bufs=1) as big,
                tc.tile_pool(name="work", bufs=3) as work,
            ):
                sb = _NS()
                sb.qT_sb = big.tile([128, 4, S], f16)           # [p, jt, s]
                sb.kT_sb = big.tile([128, 4, S], f16)
                sb.v_sb = big.tile([128, 16, HC, DK + 1], f16)  # [p, st, h, c]
                sb.wq_sb = big.tile([128, 8, JC], f16)
                sb.wk_sb = big.tile([128, 8, JC], f16)
                sb.wv_sb = big.tile([128, 8, JC], f16)
                sb.bq_sb = big.tile([128, 4], f32)
                sb.bk_sb = big.tile([128, 4], f32)
                sb.bvb_sb = big.tile([128, JC], f32)

                nc.sync.dma_start(sb.wq_sb[:], io.wqt.rearrange("(kt p) j -> p kt j", p=128))
                nc.sync.dma_start(sb.wk_sb[:], io.wkt.rearrange("(kt p) j -> p kt j", p=128))
                nc.sync.dma_start(sb.wv_sb[:], io.wvt.rearrange("(kt p) j -> p kt j", p=128))
                nc.sync.dma_start(sb.bq_sb[:], io.bq[:])
                nc.sync.dma_start(sb.bk_sb[:], io.bk[:])
                nc.sync.dma_start(sb.bvb_sb[:], io.bvb[:])

                if 1 in stages:
                    _stage1(nc, tc, work, io, sb)
                with tc.tile_pool(name="big2", bufs=1) as big2:
                    sb.ctxn_sb = big2.tile([128, 4, S], f16)    # [p, pair, s]
                    sb.wot_sb = big2.tile([128, 4, D], f16)
                    nc.sync.dma_start(
                        sb.wot_sb[:],
                        io.wot.rearrange("(kt p) j -> p kt j", p=128),
                    )
                    if 2 in stages:
                        _stage2(nc, tc, work, io, sb)
                    if 3 in stages:
                        _stage3(nc, tc, work, io, sb)

    nc.compile()
    return nc


_NC = None


def _get_nc():
    global _NC
    if _NC is None:
        _NC = build_nc()
    return _NC


def make_in_maps(Q, K, V, Wq, bq, Wk, bk, Wv, bv, Wo, bo):
    ash = lambda x: np.ascontiguousarray(np.asarray(x, dtype=np.float32).astype(np.float16))
    asf = lambda x: np.ascontiguousarray(np.asarray(x, dtype=np.float32))
    in_maps = []
    for c in range(N_CORES):
        b = c // 2
        j0 = JC * (c % 2)
        jsl = slice(j0, j0 + JC)
        in_maps.append({
            "qt": ash(np.asarray(Q)[b].T),
            "kt": ash(np.asarray(K)[b].T),
            "vt": ash(np.asarray(V)[b].T),
            "wqt": ash(np.asarray(Wq)[jsl].T),
            "wkt": ash(np.asarray(Wk)[jsl].T),
            "wvt": ash(np.asarray(Wv)[jsl].T),
            "wot": ash(np.asarray(Wo)[:, jsl].T),
            "bq": asf(np.asarray(bq)[jsl].reshape(4, 128).T),
            "bk": asf(np.asarray(bk)[jsl].reshape(4, 128).T),
            "bvb": asf(np.broadcast_to(np.asarray(bv)[jsl], (128, JC))),
        })
    return in_maps


def kernel(Q, K, V, Wq, bq, Wk, bk, Wv, bv, Wo, bo, _trace=False, _trace_kwargs=None):
    nc = _get_nc()
    in_maps = make_in_maps(Q, K, V, Wq, bq, Wk, bk, Wv, bv, Wo, bo)
    res = run_bass_kernel_spmd(
        nc, in_maps, core_ids=list(range(N_CORES)),
        trace=_trace, **(_trace_kwargs or {}),
    )
    parts = [res.results[c]["out"] for c in range(N_CORES)]
    bo_np = np.asarray(bo, dtype=np.float32)
    O = np.stack([parts[2 * b] + parts[2 * b + 1] + bo_np for b in range(4)])
    kernel.last_results = res
    return O.astype(np.float32)


# revision 10
# speedup vs baseline: 1.4353x; 1.4353x over previous
"""Multi-head attention (B=4, S=2048, D=1024, H=16) on 8 TRN2 NeuronCores.

Sharding: core c <- batch c//2, heads 8*(c%2) .. 8*(c%2)+8 (Megatron-style:
Wq/Wk/Wv column-parallel, Wo row-parallel). No collectives: the two partial
outputs per batch are summed on the host (plus the bo bias).

Per-core kernel strategy:
  - All matmul operands are fp16 (hosts converts inputs): full PE rate,
    4x cheaper LDWEIGHTS via fast-weight-load, half the DMA bytes.
  - q^T, k^T computed directly in [head_dim, seq] layout (out = W^T.T @ X^T),
    v computed in natural [seq, head_dim] layout with a ones column appended.
  - Scores computed transposed: ST[s_k, s_q] = k . q, so softmax exp is pure
    elementwise (no max subtraction needed: scores ~ N(0,1) after 1/8 scale)
    and no on-chip transposes are needed anywhere.
  - ctx^T[c, s_q] accumulated as v_aug^T @ exp(ST); the ones column yields the
    softmax denominator l[s_q] as psum row 64 for free.
  - Normalization by 1/l (reciprocal_approx_fast) before the output proj.
"""
import sys

sys.path.insert(0, "/opt/trn_rl_repo")
import numpy as np

import concourse.bass as bass
import concourse.bacc as bacc
import concourse.mybir as mybir
import concourse.tile as tile
from concourse.bass_utils import run_bass_kernel_spmd

f32 = mybir.dt.float32
f16 = mybir.dt.float16
EXP = mybir.ActivationFunctionType.Exp

S = 2048          # sequence length
D = 1024          # model dim
HC = 8            # heads per core
DK = 64           # head dim
JC = HC * DK      # per-core projection width (512)
SCALE = 0.125     # 1/sqrt(DK)
N_CORES = 8


def _stage1(nc, tc, work, io, sb):
    """QKV projections -> qT_sb, kT_sb, v_sb."""
    with (
        tc.tile_pool(name="x1", bufs=2) as xp,
        tc.tile_pool(name="ps1", bufs=3, space="PSUM") as pp,
        tc.tile_pool(name="ps1v", bufs=2, space="PSUM") as ppv,
    ):
        # q/k projections: w block [128(kt),128(j)] stationary, xT moving.
        # psum [128, 1024] per (jt, chunk) accumulated over kt (LDW:MM=1:2).
        # Order q -> v -> k so the v bias-add DVE tail drains during k-proj
        # instead of stalling the stage-2 psum pool open.
        def qk_proj(x_dram, w_sb, o_sb, b_sb):
            for sc in range(2):  # s chunks of 1024
                xq = xp.tile([128, 8, 1024], f16, tag="x")
                nc.sync.dma_start(
                    xq[:],
                    x_dram[:, sc * 1024:(sc + 1) * 1024].rearrange(
                        "(kt p) s -> p kt s", p=128
                    ),
                )
                for jt in range(4):
                    ps = pp.tile([128, 1024], f32, tag="proj")
                    for kt in range(8):
                        w = w_sb[:, kt, jt * 128:(jt + 1) * 128]
                        nc.tensor.matmul(
                            ps[:, 0:512], w, xq[:, kt, 0:512],
                            start=(kt == 0), stop=(kt == 7),
                        )
                        nc.tensor.matmul(
                            ps[:, 512:1024], w, xq[:, kt, 512:1024],
                            start=(kt == 0), stop=(kt == 7),
                        )
                    nc.vector.tensor_scalar_add(
                        o_sb[:, jt, sc * 1024:(sc + 1) * 1024],
                        ps[:],
                        b_sb[:, jt:jt + 1],
                    )

        def v_proj():
            # xT chunk stationary, wv moving; out [s, j] + bias.
            for st in range(16):
                xv = xp.tile([128, 8, 128], f16, tag="xv")
                nc.sync.dma_start(
                    xv[:],
                    io.vt[:, st * 128:(st + 1) * 128].rearrange(
                        "(kt p) s -> p kt s", p=128
                    ),
                )
                ps = ppv.tile([128, JC], f32, tag="projv")
                for kt in range(8):
                    nc.tensor.matmul(
                        ps[:],
                        xv[:, kt, :],
                        sb.wv_sb[:, kt, :],
                        start=(kt == 0), stop=(kt == 7),
                    )
                nc.vector.tensor_add(
                    sb.v_sb[:, st, :, :],
                    ps[:].rearrange("p (h c) -> p h c", h=HC),
                    sb.bvb_sb[:].rearrange("p (h c) -> p h c", h=HC),
                )

        qk_proj(io.qt, sb.wq_sb, sb.qT_sb, sb.bq_sb)
        v_proj()
        qk_proj(io.kt, sb.wk_sb, sb.kT_sb, sb.bk_sb)


def _stage23(nc, tc, work, io, sb):
    """Attention + output projection, s_q-block outer.

    Every stage-2 matmul keeps the full PE array busy (HAM stays at K=8/8):
      - scores: both heads of a pair issued back-to-back as concurrent
        row-group matmuls (K=64 at rows 0-63 / 64-127) into one [128,1024]
        psum tile (one bank per head), exp'd by a single ACTIVATE.
      - ctx and the softmax denominator: col-tiled concurrent pairs
        (M=64 at cols 0-63 / 64-127). The denominator comes from a ones
        stationary [128,64], which lands l replicated across 64 partitions
        so the reciprocal + normalize are full-width DVE ops.
    PSUM: ST 2x2 + CTX 2 + L 2 = 8 banks; stage-3 tiles rotate through the
    CTX/L tags after each s_q block.
    """
    with (
        tc.tile_pool(name="ps2st", bufs=2, space="PSUM") as pp_st,
        tc.tile_pool(name="ps2cl", bufs=1, space="PSUM") as pp_cl,
        tc.tile_pool(name="att", bufs=4) as att,
        tc.tile_pool(name="att2", bufs=2) as att2,
    ):
        for sqb in range(2):
            for pair in range(4):
                h0, h1 = 2 * pair, 2 * pair + 1
                ctxt = [pp_cl.tile([128, 512], f32, tag=f"ctx{hf}",
                                   name=f"ctx_{sqb}_{pair}_{hf}") for hf in (0, 1)]
                lt = [pp_cl.tile([128, 512], f32, tag=f"l{hf}",
                                 name=f"l_{sqb}_{pair}_{hf}") for hf in (0, 1)]

                def emit_cl(k, half, pt):
                    cx, lx = ctxt[half], lt[half]
                    st0, sp0 = (k == 0), (k == 15)
                    nc.tensor.matmul(cx[0:64, :], sb.v_sb[:, k, h0, :],
                                     pt[:, 0:512], start=st0, stop=sp0)
                    nc.tensor.matmul(cx[64:128, :], sb.v_sb[:, k, h1, :],
                                     pt[:, 512:1024], start=st0, stop=sp0)
                    nc.tensor.matmul(lx[0:64, :], sb.ones_sb[:],
                                     pt[:, 0:512], start=st0, stop=sp0)
                    nc.tensor.matmul(lx[64:128, :], sb.ones_sb[:],
                                     pt[:, 512:1024], start=st0, stop=sp0)

                pend = None
                for k in range(16):
                    for half in range(2):
                        sq0 = sqb * 1024 + half * 512
                        st = pp_st.tile([128, 1024], f32, tag="st")
                        nc.tensor.matmul(
                            st[:, 0:512],
                            sb.kT_sb[0:64, pair, k * 128:(k + 1) * 128],
                            sb.qT_sb[0:64, pair, sq0:sq0 + 512],
                            start=True, stop=True,
                        )
                        nc.tensor.matmul(
                            st[:, 512:1024],
                            sb.kT_sb[64:128, pair, k * 128:(k + 1) * 128],
                            sb.qT_sb[64:128, pair, sq0:sq0 + 512],
                            start=True, stop=True,
                        )
                        pt = att.tile([128, 1024], f16, tag="pt")
                        nc.scalar.activation(pt[:], st[:], EXP, scale=SCALE)
                        if pend is not None:
                            emit_cl(*pend)
                        pend = (k, half, pt)
                emit_cl(*pend)
                # Normalize: l is already partition-replicated, so this is
                # three full-width DVE ops per half (no gpsimd broadcast).
                for half in range(2):
                    sq0 = sqb * 1024 + half * 512
                    lc = att2.tile([128, 512], f32, tag="lc",
                                   name=f"lc_{sqb}_{pair}_{half}")
                    nc.vector.tensor_copy(lc[:], lt[half][:])
                    r = att2.tile([128, 512], f32, tag="r",
                                  name=f"r_{sqb}_{pair}_{half}")
                    nc.vector.reciprocal_approx_fast(r[:], lc[:])
                    nc.vector.tensor_mul(
                        sb.ctxn_sb[0:64, pair, sq0:sq0 + 512],
                        ctxt[half][0:64, :], r[0:64, :],
                    )
                    nc.vector.tensor_mul(
                        sb.ctxn_sb[64:128, pair, sq0:sq0 + 512],
                        ctxt[half][64:128, :], r[64:128, :],
                    )
            # Output projection for this s_q block; psum tiles rotate
            # through the ctx/l tags (same shape, slots are free now).
            for i, sq2 in enumerate(range(sqb * 8, sqb * 8 + 8)):
                for n in range(2):
                    ps = pp_cl.tile([128, 512], f32,
                                    tag=("ctx0", "ctx1", "l0", "l1")[(2 * i + n) % 4],
                                    name=f"o_{sq2}_{n}")
                    for p in range(4):
                        nc.tensor.matmul(
                            ps[:],
                            sb.ctxn_sb[:, p, sq2 * 128:(sq2 + 1) * 128],
                            sb.wot_sb[:, p, n * 512:(n + 1) * 512],
                            start=(p == 0), stop=(p == 3),
                        )
                    ob = work.tile([128, 512], f32, tag="ob")
                    nc.vector.tensor_copy(ob[:], ps[:])
                    nc.sync.dma_start(
                        io.out[sq2 * 128:(sq2 + 1) * 128, n * 512:(n + 1) * 512],
                        ob[:],
                    )


class _NS:
    pass


def build_nc(repeats=1, stages=(1, 2, 3)):
    nc = bacc.Bacc(None, target_bir_lowering=False, debug=False)

    io = _NS()
    io.qt = nc.dram_tensor("qt", [D, S], f16, kind="ExternalInput")
    io.kt = nc.dram_tensor("kt", [D, S], f16, kind="ExternalInput")
    io.vt = nc.dram_tensor("vt", [D, S], f16, kind="ExternalInput")
    io.wqt = nc.dram_tensor("wqt", [D, JC], f16, kind="ExternalInput")
    io.wkt = nc.dram_tensor("wkt", [D, JC], f16, kind="ExternalInput")
    io.wvt = nc.dram_tensor("wvt", [D, JC], f16, kind="ExternalInput")
    io.wot = nc.dram_tensor("wot", [JC, D], f16, kind="ExternalInput")
    io.bq = nc.dram_tensor("bq", [128, 4], f32, kind="ExternalInput")
    io.bk = nc.dram_tensor("bk", [128, 4], f32, kind="ExternalInput")
    io.bvb = nc.dram_tensor("bvb", [128, JC], f32, kind="ExternalInput")
    io.out = nc.dram_tensor("out", [S, D], f32, kind="ExternalOutput")

    with tile.TileContext(nc) as tc:
        for _rep in range(repeats):
            with (
                tc.tile_pool(name="big", bufs=1) as big,
                tc.tile_pool(name="work", bufs=3) as work,
            ):
                sb = _NS()
                sb.qT_sb = big.tile([128, 4, S], f16)           # [p, jt, s]
                sb.kT_sb = big.tile([128, 4, S], f16)
                sb.v_sb = big.tile([128, 16, HC, DK], f16)      # [p, st, h, c]
                sb.ones_sb = big.tile([128, DK], f16)
                sb.wq_sb = big.tile([128, 8, JC], f16)
                sb.wk_sb = big.tile([128, 8, JC], f16)
                sb.wv_sb = big.tile([128, 8, JC], f16)
                sb.bq_sb = big.tile([128, 4], f32)
                sb.bk_sb = big.tile([128, 4], f32)
                sb.bvb_sb = big.tile([128, JC], f32)

                nc.vector.memset(sb.ones_sb[:], 1.0)
                nc.sync.dma_start(sb.wq_sb[:], io.wqt.rearrange("(kt p) j -> p kt j", p=128))
                nc.sync.dma_start(sb.wk_sb[:], io.wkt.rearrange("(kt p) j -> p kt j", p=128))
                nc.sync.dma_start(sb.wv_sb[:], io.wvt.rearrange("(kt p) j -> p kt j", p=128))
                nc.sync.dma_start(sb.bq_sb[:], io.bq[:])
                nc.sync.dma_start(sb.bk_sb[:], io.bk[:])
                nc.sync.dma_start(sb.bvb_sb[:], io.bvb[:])

                if 1 in stages:
                    _stage1(nc, tc, work, io, sb)
                with tc.tile_pool(name="big2", bufs=1) as big2:
                    sb.ctxn_sb = big2.tile([128, 4, S], f16)    # [p, pair, s]
                    sb.wot_sb = big2.tile([128, 4, D], f16)
                    nc.sync.dma_start(
                        sb.wot_sb[:],
                        io.wot.rearrange("(kt p) j -> p kt j", p=128),
                    )
                    if 2 in stages:
                        _stage23(nc, tc, work, io, sb)

    nc.compile()
    return nc


_NC = None


def _get_nc():
    global _NC
    if _NC is None:
        _NC = build_nc()
    return _NC


def make_in_maps(Q, K, V, Wq, bq, Wk, bk, Wv, bv, Wo, bo):
    ash = lambda x: np.ascontiguousarray(np.asarray(x, dtype=np.float32).astype(np.float16))
    asf = lambda x: np.ascontiguousarray(np.asarray(x, dtype=np.float32))
    in_maps = []
    for c in range(N_CORES):
        b = c // 2
        j0 = JC * (c % 2)
        jsl = slice(j0, j0 + JC)
        in_maps.append({
            "qt": ash(np.asarray(Q)[b].T),
            "kt": ash(np.asarray(K)[b].T),
            "vt": ash(np.asarray(V)[b].T),
            "wqt": ash(np.asarray(Wq)[jsl].T),
            "wkt": ash(np.asarray(Wk)[jsl].T),
            "wvt": ash(np.asarray(Wv)[jsl].T),
            "wot": ash(np.asarray(Wo)[:, jsl].T),
            "bq": asf(np.asarray(bq)[jsl].reshape(4, 128).T),
            "bk": asf(np.asarray(bk)[jsl].reshape(4, 128).T),
            "bvb": asf(np.broadcast_to(np.asarray(bv)[jsl], (128, JC))),
        })
    return in_maps


def kernel(Q, K, V, Wq, bq, Wk, bk, Wv, bv, Wo, bo, _trace=False, _trace_kwargs=None):
    nc = _get_nc()
    in_maps = make_in_maps(Q, K, V, Wq, bq, Wk, bk, Wv, bv, Wo, bo)
    res = run_bass_kernel_spmd(
        nc, in_maps, core_ids=list(range(N_CORES)),
        trace=_trace, **(_trace_kwargs or {}),
    )
    parts = [res.results[c]["out"] for c in range(N_CORES)]
    bo_np = np.asarray(bo, dtype=np.float32)
    O = np.stack([parts[2 * b] + parts[2 * b + 1] + bo_np for b in range(4)])
    kernel.last_results = res
    return O.astype(np.float32)


# revision 19
# speedup vs baseline: 1.4846x; 1.0343x over previous
"""Multi-head attention (B=4, S=2048, D=1024, H=16) on 8 TRN2 NeuronCores.

Sharding: core c <- batch c//2, heads 8*(c%2) .. 8*(c%2)+8 (Megatron-style:
Wq/Wk/Wv column-parallel, Wo row-parallel). No collectives: the two partial
outputs per batch are summed on the host (plus the bo bias).

Per-core kernel strategy (all matmul operands fp16; host pre-converts):
  - The scalar engine's 256 exp ACTIVATEs ([128,1024] each, ~294us total)
    are the hard floor; everything else is arranged to hide under them.
  - Stage 2 keeps the full PE array busy so HAM stays at K=8/8:
    scores = two concurrent row-group matmuls (both heads of a pair),
    ctx and the softmax denominator = concurrent col-tiled pairs (M=64),
    with the denominator from a ones[128,64] stationary, which lands l
    replicated across 64 partitions (full-width reciprocal, no broadcast).
  - Only q/k for head-pair 0 plus v are projected up front; the remaining
    q/k projections and the first s_q block's output projection are fed
    into stage 2's tensor slack through a background-work generator.
"""
import sys

sys.path.insert(0, "/opt/trn_rl_repo")
import numpy as np

import concourse.bass as bass
import concourse.bacc as bacc
import concourse.mybir as mybir
import concourse.tile as tile
from concourse.tile import add_dep_helper
from concourse.bass_utils import run_bass_kernel_spmd

f32 = mybir.dt.float32
f16 = mybir.dt.float16
EXP = mybir.ActivationFunctionType.Exp

S = 2048          # sequence length
D = 1024          # model dim
HC = 8            # heads per core
DK = 64           # head dim
JC = HC * DK      # per-core projection width (512)
SCALE = 0.125     # 1/sqrt(DK)
N_CORES = 8


class _NS:
    pass


def build_nc():
    nc = bacc.Bacc(None, target_bir_lowering=False, debug=False)

    io = _NS()
    io.qt = nc.dram_tensor("qt", [D, S], f16, kind="ExternalInput")
    io.kt = nc.dram_tensor("kt", [D, S], f16, kind="ExternalInput")
    io.vt = nc.dram_tensor("vt", [D, S], f16, kind="ExternalInput")
    io.wqt = nc.dram_tensor("wqt", [D, JC], f16, kind="ExternalInput")
    io.wkt = nc.dram_tensor("wkt", [D, JC], f16, kind="ExternalInput")
    io.wvt = nc.dram_tensor("wvt", [D, JC], f16, kind="ExternalInput")
    io.wot = nc.dram_tensor("wot", [JC, D], f16, kind="ExternalInput")
    io.bq = nc.dram_tensor("bq", [128, 4], f32, kind="ExternalInput")
    io.bk = nc.dram_tensor("bk", [128, 4], f32, kind="ExternalInput")
    io.bvb = nc.dram_tensor("bvb", [128, JC], f32, kind="ExternalInput")
    io.out = nc.dram_tensor("out", [S, D], f32, kind="ExternalOutput")

    with tile.TileContext(nc) as tc:
        with (
            tc.tile_pool(name="big", bufs=1) as big,
            tc.tile_pool(name="work", bufs=3) as work,
            tc.tile_pool(name="xpool", bufs=1) as xp,
            tc.tile_pool(name="xvpool", bufs=2) as xvp,
            tc.tile_pool(name="att", bufs=6) as att,
            tc.tile_pool(name="att2", bufs=2) as att2,
        ):
            sb = _NS()
            sb.qT_sb = big.tile([128, 4, S], f16)           # [p, jt, s]
            sb.kT_sb = big.tile([128, 4, S], f16)
            sb.v_sb = big.tile([128, 16, HC, DK], f16)      # [p, st, h, c]
            sb.ones_sb = big.tile([128, DK], f16)
            sb.wq_sb = big.tile([128, 8, JC], f16)
            sb.wk_sb = big.tile([128, 8, JC], f16)
            sb.wv_sb = big.tile([128, 8, JC], f16)
            sb.bq_sb = big.tile([128, 4], f32)
            sb.bk_sb = big.tile([128, 4], f32)
            sb.bvb_sb = big.tile([128, JC], f32)
            sb.ctxn_sb = big.tile([128, 4, S], f16)         # [p, pair, s]
            sb.wot_sb = big.tile([128, 4, D], f16)

            nc.vector.memset(sb.ones_sb[:], 1.0)

            bias_insts = {}   # (tensor_key, jt, sc) -> bias-add instruction
            mul_insts = {}    # (sqb, pair, half) -> [mul instructions]

            def _dep(reader, writer, why):
                # Tile misses DVE-write -> matmul-stationary-read deps when
                # emission is tightly interleaved; add the edge explicitly.
                f = reader.ins if isinstance(reader, bass.BassInstruction) else reader
                t = writer.ins if isinstance(writer, bass.BassInstruction) else writer
                add_dep_helper(f, t, sync=True, reason=why)

            def dma_x(x_dram, tag):
                ts = []
                for sc in range(2):
                    t = xp.tile([128, 8, 1024], f16, tag=f"{tag}{sc}")
                    nc.sync.dma_start(
                        t[:],
                        x_dram[:, sc * 1024:(sc + 1) * 1024].rearrange(
                            "(kt p) s -> p kt s", p=128
                        ),
                    )
                    ts.append(t)
                return ts

            # --- input DMAs, in consumption order -------------------------
            nc.sync.dma_start(sb.wq_sb[:], io.wqt.rearrange("(kt p) j -> p kt j", p=128))
            nc.sync.dma_start(sb.bq_sb[:], io.bq[:])
            xq = dma_x(io.qt, "xq")
            nc.sync.dma_start(sb.wk_sb[:], io.wkt.rearrange("(kt p) j -> p kt j", p=128))
            nc.sync.dma_start(sb.bk_sb[:], io.bk[:])
            xk = dma_x(io.kt, "xk")
            nc.sync.dma_start(sb.wv_sb[:], io.wvt.rearrange("(kt p) j -> p kt j", p=128))
            nc.sync.dma_start(sb.bvb_sb[:], io.bvb[:])
            nc.sync.dma_start(sb.wot_sb[:], io.wot.rearrange("(kt p) j -> p kt j", p=128))

            def qk_fill(pool, tag, key, x_ts, w_sb, o_sb, b_sb, jt, sc):
                """One [128,1024] projection fill: 16 MMs + bias add."""
                ps = pool.tile([128, 1024], f32, tag=tag, name=f"pj_{key}_{jt}_{sc}")
                for kt in range(8):
                    w = w_sb[:, kt, jt * 128:(jt + 1) * 128]
                    nc.tensor.matmul(ps[:, 0:512], w, x_ts[sc][:, kt, 0:512],
                                     start=(kt == 0), stop=(kt == 7))
                    nc.tensor.matmul(ps[:, 512:1024], w, x_ts[sc][:, kt, 512:1024],
                                     start=(kt == 0), stop=(kt == 7))
                    yield
                bias_insts[(key, jt, sc)] = nc.vector.tensor_scalar_add(
                    o_sb[:, jt, sc * 1024:(sc + 1) * 1024], ps[:], b_sb[:, jt:jt + 1])
                yield

            def run(gen):
                for _ in gen:
                    pass

            # --- upfront: q/k for pair 0, all of v ------------------------
            with (
                tc.tile_pool(name="fat", bufs=2, space="PSUM") as fat,
                tc.tile_pool(name="fatv", bufs=2, space="PSUM") as fatv,
            ):
                run(qk_fill(fat, "proj", "q", xq, sb.wq_sb, sb.qT_sb, sb.bq_sb, 0, 0))
                run(qk_fill(fat, "proj", "q", xq, sb.wq_sb, sb.qT_sb, sb.bq_sb, 0, 1))
                run(qk_fill(fat, "proj", "k", xk, sb.wk_sb, sb.kT_sb, sb.bk_sb, 0, 0))
                run(qk_fill(fat, "proj", "k", xk, sb.wk_sb, sb.kT_sb, sb.bk_sb, 0, 1))
                for st in range(16):
                    xv = xvp.tile([128, 8, 128], f16, tag="xv")
                    nc.sync.dma_start(
                        xv[:],
                        io.vt[:, st * 128:(st + 1) * 128].rearrange(
                            "(kt p) s -> p kt s", p=128
                        ),
                    )
                    ps = fatv.tile([128, JC], f32, tag="projv")
                    for kt in range(8):
                        nc.tensor.matmul(ps[:], xv[:, kt, :], sb.wv_sb[:, kt, :],
                                         start=(kt == 0), stop=(kt == 7))
                    nc.vector.tensor_add(
                        sb.v_sb[:, st, :, :],
                        ps[:].rearrange("p (h c) -> p h c", h=HC),
                        sb.bvb_sb[:].rearrange("p (h c) -> p h c", h=HC),
                    )

            # --- stage 2/3 with background stage-1 work -------------------
            with (
                tc.tile_pool(name="ps2st", bufs=2, space="PSUM") as pp_st,
                tc.tile_pool(name="ps2cl", bufs=1, space="PSUM") as pp_cl,
                tc.tile_pool(name="pj", bufs=1, space="PSUM") as pp_pj,
            ):
                def stage3_chunk(sq2, tagit):
                    sqb_r, half_r = sq2 // 8, (sq2 % 8) // 4
                    for n in range(2):
                        ps = pp_pj.tile([128, 1024], f32, tag=next(tagit),
                                        name=f"o_{sq2}_{n}")
                        for p in range(4):
                            omm = nc.tensor.matmul(
                                ps[:, 0:512],
                                sb.ctxn_sb[:, p, sq2 * 128:(sq2 + 1) * 128],
                                sb.wot_sb[:, p, n * 512:(n + 1) * 512],
                                start=(p == 0), stop=(p == 3),
                            )
                            if n == 0:
                                for m in mul_insts[(sqb_r, p, half_r)]:
                                    _dep(omm, m, f"out({sq2}) after ctxn")
                            if p % 2:
                                yield
                        ob = work.tile([128, 512], f32, tag="ob")
                        nc.vector.tensor_copy(ob[:], ps[:, 0:512])
                        nc.sync.dma_start(
                            io.out[sq2 * 128:(sq2 + 1) * 128,
                                   n * 512:(n + 1) * 512],
                            ob[:],
                        )
                        yield

                def bg_qk():
                    # ordered by deadline: pair p (group 32p) needs q-sc0,
                    # k-sc0, k-sc1 of jt=p; the q-sc1 fills are only read in
                    # s_q block 1 and trail behind.
                    q_args = ("q", xq, sb.wq_sb, sb.qT_sb, sb.bq_sb)
                    k_args = ("k", xk, sb.wk_sb, sb.kT_sb, sb.bk_sb)
                    for jt in range(1, 4):
                        for (key, x_ts, w_sb, o_sb, b_sb), sc in (
                            (q_args, 0), (k_args, 0), (k_args, 1),
                        ):
                            yield from qk_fill(pp_pj, "pj", key, x_ts, w_sb,
                                               o_sb, b_sb, jt, sc)
                    for jt in range(1, 4):
                        key, x_ts, w_sb, o_sb, b_sb = q_args
                        yield from qk_fill(pp_pj, "pj", key, x_ts, w_sb,
                                           o_sb, b_sb, jt, 1)

                def bg_s3():
                    # output projection for s_q block 0 (runs during block 1)
                    import itertools
                    tagit = itertools.cycle(["pj"])
                    for sq2 in range(8):
                        yield from stage3_chunk(sq2, tagit)

                bgs = [bg_qk()]
                DISABLE_INTERLEAVE = False
                S3_IN_BG = True

                def pump():
                    if DISABLE_INTERLEAVE:
                        return
                    while bgs:
                        try:
                            next(bgs[0])
                            return
                        except StopIteration:
                            bgs.pop(0)

                if DISABLE_INTERLEAVE:
                    while bgs:
                        try:
                            next(bgs[0])
                        except StopIteration:
                            bgs.pop(0)

                for sqb in range(2):
                    if sqb == 1 and not DISABLE_INTERLEAVE and S3_IN_BG:
                        bgs.append(bg_s3())
                    for pair in range(4):
                        h0, h1 = 2 * pair, 2 * pair + 1
                        for half in range(2):
                            sq0 = sqb * 1024 + half * 512
                            ctx = pp_cl.tile([128, 512], f32, tag="ctx",
                                             name=f"ctx_{sqb}_{pair}_{half}")
                            lx = pp_cl.tile([128, 512], f32, tag="l",
                                            name=f"l_{sqb}_{pair}_{half}")

                            def emit_cl(k, pt):
                                st0, sp0 = (k == 0), (k == 15)
                                nc.tensor.matmul(ctx[0:64, :], sb.v_sb[:, k, h0, :],
                                                 pt[:, 0:512], start=st0, stop=sp0,
                                                 skip_group_check=True)
                                nc.tensor.matmul(ctx[64:128, :], sb.v_sb[:, k, h1, :],
                                                 pt[:, 512:1024], start=st0, stop=sp0,
                                                 skip_group_check=True)
                                nc.tensor.matmul(lx[0:64, :], sb.ones_sb[:],
                                                 pt[:, 0:512], start=st0, stop=sp0,
                                                 skip_group_check=True)
                                nc.tensor.matmul(lx[64:128, :], sb.ones_sb[:],
                                                 pt[:, 512:1024], start=st0, stop=sp0,
                                                 skip_group_check=True)

                            pend = None
                            for k in range(16):
                                st = pp_st.tile([128, 1024], f32, tag="st")
                                smm = nc.tensor.matmul(
                                    st[:, 0:512],
                                    sb.kT_sb[0:64, pair, k * 128:(k + 1) * 128],
                                    sb.qT_sb[0:64, pair, sq0:sq0 + 512],
                                    start=True, stop=True,
                                )
                                if half == 0 and k in (0, 8) and pair + sqb > 0:
                                    why = f"scores({sqb},{pair}) after qk bias"
                                    if k == 0:
                                        _dep(smm, bias_insts[("q", pair, sqb)], why)
                                        _dep(smm, bias_insts[("k", pair, 0)], why)
                                    else:
                                        _dep(smm, bias_insts[("k", pair, 1)], why)
                                nc.tensor.matmul(
                                    st[:, 512:1024],
                                    sb.kT_sb[64:128, pair, k * 128:(k + 1) * 128],
                                    sb.qT_sb[64:128, pair, sq0:sq0 + 512],
                                    start=True, stop=True,
                                )
                                pt = att.tile([128, 1024], f16, tag="pt")
                                nc.scalar.activation(pt[:], st[:], EXP, scale=SCALE)
                                if pend is not None:
                                    emit_cl(*pend)
                                pump()
                                pend = (k, pt)
                            emit_cl(*pend)
                            # normalize (l is partition-replicated already)
                            lc = att2.tile([128, 512], f32, tag="lc",
                                           name=f"lc_{sqb}_{pair}_{half}")
                            nc.vector.tensor_copy(lc[:], lx[:])
                            r = att2.tile([128, 512], f32, tag="r",
                                          name=f"r_{sqb}_{pair}_{half}")
                            nc.vector.reciprocal_approx_fast(r[:], lc[:])
                            mul_insts[(sqb, pair, half)] = [
                                nc.vector.tensor_mul(
                                    sb.ctxn_sb[0:64, pair, sq0:sq0 + 512],
                                    ctx[0:64, :], r[0:64, :],
                                ),
                                nc.vector.tensor_mul(
                                    sb.ctxn_sb[64:128, pair, sq0:sq0 + 512],
                                    ctx[64:128, :], r[64:128, :],
                                ),
                            ]

                # drain any remaining background work
                while bgs:
                    try:
                        next(bgs[0])
                    except StopIteration:
                        bgs.pop(0)
                if DISABLE_INTERLEAVE or not S3_IN_BG:
                    for _ in bg_s3():
                        pass
                # output projection for s_q block 1 (tail)
                import itertools
                tagit = itertools.cycle(["pj", "ctx", "l"])

                def stage3_tail(sq2):
                    for n in range(2):
                        tg2 = next(tagit)
                        ps = (pp_pj if tg2 == "pj" else pp_cl).tile(
                            [128, 1024] if tg2 == "pj" else [128, 512],
                            f32, tag=tg2, name=f"o_{sq2}_{n}")
                        for p in range(4):
                            omm = nc.tensor.matmul(
                                ps[:, 0:512],
                                sb.ctxn_sb[:, p, sq2 * 128:(sq2 + 1) * 128],
                                sb.wot_sb[:, p, n * 512:(n + 1) * 512],
                                start=(p == 0), stop=(p == 3),
                            )
                            if n == 0:
                                for m in mul_insts[(sq2 // 8, p, (sq2 % 8) // 4)]:
                                    _dep(omm, m, f"out({sq2}) after ctxn")
                        ob = work.tile([128, 512], f32, tag="ob")
                        nc.vector.tensor_copy(ob[:], ps[:, 0:512])
                        nc.sync.dma_start(
                            io.out[sq2 * 128:(sq2 + 1) * 128,
                                   n * 512:(n + 1) * 512],
                            ob[:],
                        )

                for sq2 in range(8, 16):
                    stage3_tail(sq2)

    nc.compile()
    return nc


_NC = None


def _get_nc():
    global _NC
    if _NC is None:
        _NC = build_nc()
    return _NC


def make_in_maps(Q, K, V, Wq, bq, Wk, bk, Wv, bv, Wo, bo):
    ash = lambda x: np.ascontiguousarray(np.asarray(x, dtype=np.float32).astype(np.float16))
    asf = lambda x: np.ascontiguousarray(np.asarray(x, dtype=np.float32))
    in_maps = []
    for c in range(N_CORES):
        b = c // 2
        j0 = JC * (c % 2)
        jsl = slice(j0, j0 + JC)
        in_maps.append({
            "qt": ash(np.asarray(Q)[b].T),
            "kt": ash(np.asarray(K)[b].T),
            "vt": ash(np.asarray(V)[b].T),
            "wqt": ash(np.asarray(Wq)[jsl].T),
            "wkt": ash(np.asarray(Wk)[jsl].T),
            "wvt": ash(np.asarray(Wv)[jsl].T),
            "wot": ash(np.asarray(Wo)[:, jsl].T),
            "bq": asf(np.asarray(bq)[jsl].reshape(4, 128).T),
            "bk": asf(np.asarray(bk)[jsl].reshape(4, 128).T),
            "bvb": asf(np.broadcast_to(np.asarray(bv)[jsl], (128, JC))),
        })
    return in_maps


def kernel(Q, K, V, Wq, bq, Wk, bk, Wv, bv, Wo, bo, _trace=False, _trace_kwargs=None):
    nc = _get_nc()
    in_maps = make_in_maps(Q, K, V, Wq, bq, Wk, bk, Wv, bv, Wo, bo)
    res = run_bass_kernel_spmd(
        nc, in_maps, core_ids=list(range(N_CORES)),
        trace=_trace, **(_trace_kwargs or {}),
    )
    parts = [res.results[c]["out"] for c in range(N_CORES)]
    bo_np = np.asarray(bo, dtype=np.float32)
    O = np.stack([parts[2 * b] + parts[2 * b + 1] + bo_np for b in range(4)])
    kernel.last_results = res
    return O.astype(np.float32)


# revision 21
# speedup vs baseline: 1.5638x; 1.0534x over previous
"""Multi-head attention (B=4, S=2048, D=1024, H=16) on 8 TRN2 NeuronCores.

Sharding: core c <- batch c//2, heads 8*(c%2) .. 8*(c%2)+8 (Megatron-style:
Wq/Wk/Wv column-parallel, Wo row-parallel). No collectives: the two partial
outputs per batch are summed on the host (plus the bo bias).

Per-core kernel strategy (all matmul operands fp16; host pre-converts):
  - The scalar engine's 256 exp ACTIVATEs ([128,1024] each, ~294us total)
    are the hard floor; everything else is arranged to hide under them.
  - Stage 2 keeps the full PE array busy so HAM stays at K=8/8:
    scores = two concurrent row-group matmuls (both heads of a pair),
    ctx and the softmax denominator = concurrent col-tiled pairs (M=64),
    with the denominator from a ones[128,64] stationary, which lands l
    replicated across 64 partitions (full-width reciprocal, no broadcast).
  - Only q/k for head-pair 0 plus v are projected up front; the remaining
    q/k projections and the first s_q block's output projection are fed
    into stage 2's tensor slack through a background-work generator.
"""
import sys

sys.path.insert(0, "/opt/trn_rl_repo")
import numpy as np

import concourse.bass as bass
import concourse.bacc as bacc
import concourse.mybir as mybir
import concourse.tile as tile
from concourse.tile import add_dep_helper
from concourse.bass_utils import run_bass_kernel_spmd

f32 = mybir.dt.float32
f16 = mybir.dt.float16
EXP = mybir.ActivationFunctionType.Exp

S = 2048          # sequence length
D = 1024          # model dim
HC = 8            # heads per core
DK = 64           # head dim
JC = HC * DK      # per-core projection width (512)
SCALE = 0.125     # 1/sqrt(DK)
N_CORES = 8


class _NS:
    pass


def build_nc():
    nc = bacc.Bacc(None, target_bir_lowering=False, debug=False)

    io = _NS()
    io.qt = nc.dram_tensor("qt", [D, S], f16, kind="ExternalInput")
    io.kt = nc.dram_tensor("kt", [D, S], f16, kind="ExternalInput")
    io.vt = nc.dram_tensor("vt", [D, S], f16, kind="ExternalInput")
    io.wqt = nc.dram_tensor("wqt", [D, JC], f16, kind="ExternalInput")
    io.wkt = nc.dram_tensor("wkt", [D, JC], f16, kind="ExternalInput")
    io.wvt = nc.dram_tensor("wvt", [D, JC], f16, kind="ExternalInput")
    io.wot = nc.dram_tensor("wot", [JC, D], f16, kind="ExternalInput")
    io.bq = nc.dram_tensor("bq", [128, 4], f32, kind="ExternalInput")
    io.bk = nc.dram_tensor("bk", [128, 4], f32, kind="ExternalInput")
    io.bvb = nc.dram_tensor("bvb", [128, JC], f32, kind="ExternalInput")
    io.out = nc.dram_tensor("out", [S, D], f16, kind="ExternalOutput")

    with tile.TileContext(nc) as tc:
        with (
            tc.tile_pool(name="big", bufs=1) as big,
            tc.tile_pool(name="work", bufs=3) as work,
            tc.tile_pool(name="xpool", bufs=1) as xp,
            tc.tile_pool(name="xvpool", bufs=4) as xvp,
            tc.tile_pool(name="att", bufs=6) as att,
            tc.tile_pool(name="att2", bufs=2) as att2,
        ):
            sb = _NS()
            sb.qT_sb = big.tile([128, 4, S], f16)           # [p, jt, s]
            sb.kT_sb = big.tile([128, 4, S], f16)
            sb.v_sb = big.tile([128, 16, HC, DK], f16)      # [p, st, h, c]
            sb.ones_sb = big.tile([128, DK], f16)
            sb.wq_sb = big.tile([128, 8, JC], f16)
            sb.wk_sb = big.tile([128, 8, JC], f16)
            sb.wv_sb = big.tile([128, 8, JC], f16)
            sb.bq_sb = big.tile([128, 4], f32)
            sb.bk_sb = big.tile([128, 4], f32)
            sb.bvb_sb = big.tile([128, JC], f32)
            sb.ctxn_sb = big.tile([128, 4, S], f16)         # [p, pair, s]
            sb.wot_sb = big.tile([128, 4, D], f16)

            nc.vector.memset(sb.ones_sb[:], 1.0)

            bias_insts = {}   # (tensor_key, jt, sc) -> bias-add instruction
            mul_insts = {}    # (sqb, pair, half) -> [mul instructions]

            def _dep(reader, writer, why):
                # Tile misses DVE-write -> matmul-stationary-read deps when
                # emission is tightly interleaved; add the edge explicitly.
                f = reader.ins if isinstance(reader, bass.BassInstruction) else reader
                t = writer.ins if isinstance(writer, bass.BassInstruction) else writer
                add_dep_helper(f, t, sync=True, reason=why)

            def dma_x(x_dram, tag, chunks, ts=None):
                ts = [None, None] if ts is None else ts
                for sc in chunks:
                    t = xp.tile([128, 8, 1024], f16, tag=f"{tag}{sc}")
                    nc.sync.dma_start(
                        t[:],
                        x_dram[:, sc * 1024:(sc + 1) * 1024].rearrange(
                            "(kt p) s -> p kt s", p=128
                        ),
                    )
                    ts[sc] = t
                return ts

            # --- input DMAs: v first (ctx needs it almost immediately) ----
            nc.sync.dma_start(sb.wv_sb[:], io.wvt.rearrange("(kt p) j -> p kt j", p=128))
            nc.sync.dma_start(sb.bvb_sb[:], io.bvb[:])

            def qk_fill(pool, tag, key, x_ts, w_sb, o_sb, b_sb, jt, sc):
                """One [128,1024] projection fill: 16 MMs + bias add."""
                ps = pool.tile([128, 1024], f32, tag=tag, name=f"pj_{key}_{jt}_{sc}")
                for kt in range(8):
                    w = w_sb[:, kt, jt * 128:(jt + 1) * 128]
                    nc.tensor.matmul(ps[:, 0:512], w, x_ts[sc][:, kt, 0:512],
                                     start=(kt == 0), stop=(kt == 7))
                    nc.tensor.matmul(ps[:, 512:1024], w, x_ts[sc][:, kt, 512:1024],
                                     start=(kt == 0), stop=(kt == 7))
                    yield
                bias_insts[(key, jt, sc)] = nc.vector.tensor_scalar_add(
                    o_sb[:, jt, sc * 1024:(sc + 1) * 1024], ps[:], b_sb[:, jt:jt + 1])
                yield

            def run(gen):
                for _ in gen:
                    pass

            # --- upfront: all of v, then q-jt0-c0, k-jt0-c0/c1 ------------
            with (
                tc.tile_pool(name="fat", bufs=2, space="PSUM") as fat,
                tc.tile_pool(name="fatv", bufs=2, space="PSUM") as fatv,
            ):
                for st in range(16):
                    xv = xvp.tile([128, 8, 128], f16, tag="xv")
                    nc.sync.dma_start(
                        xv[:],
                        io.vt[:, st * 128:(st + 1) * 128].rearrange(
                            "(kt p) s -> p kt s", p=128
                        ),
                    )
                    ps = fatv.tile([128, JC], f32, tag="projv")
                    for kt in range(8):
                        nc.tensor.matmul(ps[:], xv[:, kt, :], sb.wv_sb[:, kt, :],
                                         start=(kt == 0), stop=(kt == 7))
                    nc.vector.tensor_add(
                        sb.v_sb[:, st, :, :],
                        ps[:].rearrange("p (h c) -> p h c", h=HC),
                        sb.bvb_sb[:].rearrange("p (h c) -> p h c", h=HC),
                    )
                nc.sync.dma_start(sb.wq_sb[:], io.wqt.rearrange("(kt p) j -> p kt j", p=128))
                nc.sync.dma_start(sb.bq_sb[:], io.bq[:])
                xq = dma_x(io.qt, "xq", (0,))
                nc.sync.dma_start(sb.wk_sb[:], io.wkt.rearrange("(kt p) j -> p kt j", p=128))
                nc.sync.dma_start(sb.bk_sb[:], io.bk[:])
                xk = dma_x(io.kt, "xk", (0, 1))
                run(qk_fill(fat, "proj", "q", xq, sb.wq_sb, sb.qT_sb, sb.bq_sb, 0, 0))
                run(qk_fill(fat, "proj", "k", xk, sb.wk_sb, sb.kT_sb, sb.bk_sb, 0, 0))
                run(qk_fill(fat, "proj", "k", xk, sb.wk_sb, sb.kT_sb, sb.bk_sb, 0, 1))
                # xq chunk 1 DMA + wot can trail (needed in s_q block 1)
                dma_x(io.qt, "xq", (1,), ts=xq)
                nc.sync.dma_start(sb.wot_sb[:], io.wot.rearrange("(kt p) j -> p kt j", p=128))

            # --- stage 2/3 with background stage-1 work -------------------
            with (
                tc.tile_pool(name="ps2st", bufs=2, space="PSUM") as pp_st,
                tc.tile_pool(name="ps2cl", bufs=1, space="PSUM") as pp_cl,
                tc.tile_pool(name="pj", bufs=1, space="PSUM") as pp_pj,
            ):
                def stage3_chunk(sq2, tagit):
                    sqb_r, half_r = sq2 // 8, (sq2 % 8) // 4
                    for n in range(2):
                        ps = pp_pj.tile([128, 1024], f32, tag=next(tagit),
                                        name=f"o_{sq2}_{n}")
                        for p in range(4):
                            omm = nc.tensor.matmul(
                                ps[:, 0:512],
                                sb.ctxn_sb[:, p, sq2 * 128:(sq2 + 1) * 128],
                                sb.wot_sb[:, p, n * 512:(n + 1) * 512],
                                start=(p == 0), stop=(p == 3),
                            )
                            if n == 0:
                                for m in mul_insts[(sqb_r, p, half_r)]:
                                    _dep(omm, m, f"out({sq2}) after ctxn")
                            if p % 2:
                                yield
                        ob = work.tile([128, 512], f16, tag="ob")
                        nc.vector.tensor_copy(ob[:], ps[:, 0:512])
                        nc.sync.dma_start(
                            io.out[sq2 * 128:(sq2 + 1) * 128,
                                   n * 512:(n + 1) * 512],
                            ob[:],
                        )
                        yield

                def bg_qk():
                    # ordered by deadline: pair p (group 32p) needs q-sc0,
                    # k-sc0, k-sc1 of jt=p; the q-sc1 fills are only read in
                    # s_q block 1 and trail behind.
                    q_args = ("q", xq, sb.wq_sb, sb.qT_sb, sb.bq_sb)
                    k_args = ("k", xk, sb.wk_sb, sb.kT_sb, sb.bk_sb)
                    for jt in range(1, 4):
                        for (key, x_ts, w_sb, o_sb, b_sb), sc in (
                            (q_args, 0), (k_args, 0), (k_args, 1),
                        ):
                            yield from qk_fill(pp_pj, "pj", key, x_ts, w_sb,
                                               o_sb, b_sb, jt, sc)
                    for jt in range(0, 4):
                        key, x_ts, w_sb, o_sb, b_sb = q_args
                        yield from qk_fill(pp_pj, "pj", key, x_ts, w_sb,
                                           o_sb, b_sb, jt, 1)

                def bg_s3():
                    # output projection for s_q block 0 (runs during block 1)
                    import itertools
                    tagit = itertools.cycle(["pj"])
                    for sq2 in range(8):
                        yield from stage3_chunk(sq2, tagit)

                def bg_s3b():
                    # block-1 rows whose ctxn (half 0) is already complete
                    import itertools
                    tagit = itertools.cycle(["pj"])
                    for sq2 in range(8, 12):
                        yield from stage3_chunk(sq2, tagit)

                bgs = [bg_qk()]
                DISABLE_INTERLEAVE = False
                S3_IN_BG = True

                def pump():
                    if DISABLE_INTERLEAVE:
                        return
                    while bgs:
                        try:
                            next(bgs[0])
                            return
                        except StopIteration:
                            bgs.pop(0)

                if DISABLE_INTERLEAVE:
                    while bgs:
                        try:
                            next(bgs[0])
                        except StopIteration:
                            bgs.pop(0)

                for sqb in range(2):
                    if sqb == 1 and not DISABLE_INTERLEAVE and S3_IN_BG:
                        bgs.append(bg_s3())
                    for pair in range(4):
                        h0, h1 = 2 * pair, 2 * pair + 1
                        for half in range(2):
                            if sqb == 1 and pair == 3 and half == 1:
                                bgs.append(bg_s3b())
                            sq0 = sqb * 1024 + half * 512
                            ctx = pp_cl.tile([128, 512], f32, tag="ctx",
                                             name=f"ctx_{sqb}_{pair}_{half}")
                            lx = pp_cl.tile([128, 512], f32, tag="l",
                                            name=f"l_{sqb}_{pair}_{half}")

                            def emit_cl(k, pt):
                                st0, sp0 = (k == 0), (k == 15)
                                nc.tensor.matmul(ctx[0:64, :], sb.v_sb[:, k, h0, :],
                                                 pt[:, 0:512], start=st0, stop=sp0,
                                                 skip_group_check=True)
                                nc.tensor.matmul(ctx[64:128, :], sb.v_sb[:, k, h1, :],
                                                 pt[:, 512:1024], start=st0, stop=sp0,
                                                 skip_group_check=True)
                                nc.tensor.matmul(lx[0:64, :], sb.ones_sb[:],
                                                 pt[:, 0:512], start=st0, stop=sp0,
                                                 skip_group_check=True)
                                nc.tensor.matmul(lx[64:128, :], sb.ones_sb[:],
                                                 pt[:, 512:1024], start=st0, stop=sp0,
                                                 skip_group_check=True)

                            pend = None
                            for k in range(16):
                                st = pp_st.tile([128, 1024], f32, tag="st")
                                smm = nc.tensor.matmul(
                                    st[:, 0:512],
                                    sb.kT_sb[0:64, pair, k * 128:(k + 1) * 128],
                                    sb.qT_sb[0:64, pair, sq0:sq0 + 512],
                                    start=True, stop=True,
                                )
                                if half == 0 and k in (0, 8) and pair + sqb > 0:
                                    why = f"scores({sqb},{pair}) after qk bias"
                                    if k == 0:
                                        _dep(smm, bias_insts[("q", pair, sqb)], why)
                                        _dep(smm, bias_insts[("k", pair, 0)], why)
                                    else:
                                        _dep(smm, bias_insts[("k", pair, 1)], why)
                                nc.tensor.matmul(
                                    st[:, 512:1024],
                                    sb.kT_sb[64:128, pair, k * 128:(k + 1) * 128],
                                    sb.qT_sb[64:128, pair, sq0:sq0 + 512],
                                    start=True, stop=True,
                                )
                                pt = att.tile([128, 1024], f16, tag="pt")
                                nc.scalar.activation(pt[:], st[:], EXP, scale=SCALE)
                                if pend is not None:
                                    emit_cl(*pend)
                                pump()
                                pend = (k, pt)
                            emit_cl(*pend)
                            # normalize (l is partition-replicated already)
                            lc = att2.tile([128, 512], f32, tag="lc",
                                           name=f"lc_{sqb}_{pair}_{half}")
                            nc.vector.tensor_copy(lc[:], lx[:])
                            r = att2.tile([128, 512], f32, tag="r",
                                          name=f"r_{sqb}_{pair}_{half}")
                            nc.vector.reciprocal_approx_fast(r[:], lc[:])
                            mul_insts[(sqb, pair, half)] = [
                                nc.vector.tensor_mul(
                                    sb.ctxn_sb[0:64, pair, sq0:sq0 + 512],
                                    ctx[0:64, :], r[0:64, :],
                                ),
                                nc.vector.tensor_mul(
                                    sb.ctxn_sb[64:128, pair, sq0:sq0 + 512],
                                    ctx[64:128, :], r[64:128, :],
                                ),
                            ]

                # drain any remaining background work
                while bgs:
                    try:
                        next(bgs[0])
                    except StopIteration:
                        bgs.pop(0)
                if DISABLE_INTERLEAVE or not S3_IN_BG:
                    for _ in bg_s3():
                        pass
                # output projection for s_q block 1 (tail)
                import itertools
                tagit = itertools.cycle(["pj", "ctx", "l"])

                def stage3_tail(sq2):
                    for n in range(2):
                        tg2 = next(tagit)
                        ps = (pp_pj if tg2 == "pj" else pp_cl).tile(
                            [128, 1024] if tg2 == "pj" else [128, 512],
                            f32, tag=tg2, name=f"o_{sq2}_{n}")
                        for p in range(4):
                            omm = nc.tensor.matmul(
                                ps[:, 0:512],
                                sb.ctxn_sb[:, p, sq2 * 128:(sq2 + 1) * 128],
                                sb.wot_sb[:, p, n * 512:(n + 1) * 512],
                                start=(p == 0), stop=(p == 3),
                            )
                            if n == 0:
                                for m in mul_insts[(sq2 // 8, p, (sq2 % 8) // 4)]:
                                    _dep(omm, m, f"out({sq2}) after ctxn")
                        ob = work.tile([128, 512], f16, tag="ob")
                        nc.vector.tensor_copy(ob[:], ps[:, 0:512])
                        nc.sync.dma_start(
                            io.out[sq2 * 128:(sq2 + 1) * 128,
                                   n * 512:(n + 1) * 512],
                            ob[:],
                        )

                for sq2 in range(12, 16):
                    stage3_tail(sq2)

    nc.compile()
    return nc


_NC = None


def _get_nc():
    global _NC
    if _NC is None:
        _NC = build_nc()
    return _NC


def make_in_maps(Q, K, V, Wq, bq, Wk, bk, Wv, bv, Wo, bo):
    ash = lambda x: np.ascontiguousarray(np.asarray(x, dtype=np.float32).astype(np.float16))
    asf = lambda x: np.ascontiguousarray(np.asarray(x, dtype=np.float32))
    in_maps = []
    for c in range(N_CORES):
        b = c // 2
        j0 = JC * (c % 2)
        jsl = slice(j0, j0 + JC)
        in_maps.append({
            "qt": ash(np.asarray(Q)[b].T),
            "kt": ash(np.asarray(K)[b].T),
            "vt": ash(np.asarray(V)[b].T),
            "wqt": ash(np.asarray(Wq)[jsl].T),
            "wkt": ash(np.asarray(Wk)[jsl].T),
            "wvt": ash(np.asarray(Wv)[jsl].T),
            "wot": ash(np.asarray(Wo)[:, jsl].T),
            "bq": asf(np.asarray(bq)[jsl].reshape(4, 128).T),
            "bk": asf(np.asarray(bk)[jsl].reshape(4, 128).T),
            "bvb": asf(np.broadcast_to(np.asarray(bv)[jsl], (128, JC))),
        })
    return in_maps


def kernel(Q, K, V, Wq, bq, Wk, bk, Wv, bv, Wo, bo, _trace=False, _trace_kwargs=None):
    nc = _get_nc()
    in_maps = make_in_maps(Q, K, V, Wq, bq, Wk, bk, Wv, bv, Wo, bo)
    res = run_bass_kernel_spmd(
        nc, in_maps, core_ids=list(range(N_CORES)),
        trace=_trace, **(_trace_kwargs or {}),
    )
    parts = [res.results[c]["out"].astype(np.float32) for c in range(N_CORES)]
    bo_np = np.asarray(bo, dtype=np.float32)
    O = np.stack([parts[2 * b] + parts[2 * b + 1] + bo_np for b in range(4)])
    kernel.last_results = res
    return O.astype(np.float32)


# revision 22
# speedup vs baseline: 1.5664x; 1.0016x over previous
"""Multi-head attention (B=4, S=2048, D=1024, H=16) on 8 TRN2 NeuronCores.

Sharding: core c <- batch c//2, heads 8*(c%2) .. 8*(c%2)+8 (Megatron-style:
Wq/Wk/Wv column-parallel, Wo row-parallel). No collectives: the two partial
outputs per batch are summed on the host (plus the bo bias).

Per-core kernel strategy (all matmul operands fp16; host pre-converts):
  - The scalar engine's 256 exp ACTIVATEs ([128,1024] each, ~294us total)
    are the hard floor; everything else is arranged to hide under them.
  - Stage 2 keeps the full PE array busy so HAM stays at K=8/8:
    scores = two concurrent row-group matmuls (both heads of a pair),
    ctx and the softmax denominator = concurrent col-tiled pairs (M=64),
    with the denominator from a ones[128,64] stationary, which lands l
    replicated across 64 partitions (full-width reciprocal, no broadcast).
  - v is projected first (ctx needs it almost immediately); q/k for
    head-pair 0 follow; the remaining q/k projections and most of the
    output projection are fed into stage 2's tensor slack through a
    background-work generator, with explicit add_dep_helper edges at the
    DVE-write -> matmul-stationary-read hazards Tile misses.
"""
import itertools
import sys

sys.path.insert(0, "/opt/trn_rl_repo")
import numpy as np

import concourse.bass as bass
import concourse.bacc as bacc
import concourse.mybir as mybir
import concourse.tile as tile
from concourse.tile import add_dep_helper
from concourse.bass_utils import run_bass_kernel_spmd

f32 = mybir.dt.float32
f16 = mybir.dt.float16
EXP = mybir.ActivationFunctionType.Exp

S = 2048          # sequence length
D = 1024          # model dim
HC = 8            # heads per core
DK = 64           # head dim
JC = HC * DK      # per-core projection width (512)
SCALE = 0.125     # 1/sqrt(DK)
N_CORES = 8


class _NS:
    pass


def build_nc():
    nc = bacc.Bacc(None, target_bir_lowering=False, debug=False)

    io = _NS()
    io.qt = nc.dram_tensor("qt", [D, S], f16, kind="ExternalInput")
    io.kt = nc.dram_tensor("kt", [D, S], f16, kind="ExternalInput")
    io.vt = nc.dram_tensor("vt", [D, S], f16, kind="ExternalInput")
    io.wqt = nc.dram_tensor("wqt", [D, JC], f16, kind="ExternalInput")
    io.wkt = nc.dram_tensor("wkt", [D, JC], f16, kind="ExternalInput")
    io.wvt = nc.dram_tensor("wvt", [D, JC], f16, kind="ExternalInput")
    io.wot = nc.dram_tensor("wot", [JC, D], f16, kind="ExternalInput")
    io.bq = nc.dram_tensor("bq", [128, 4], f32, kind="ExternalInput")
    io.bk = nc.dram_tensor("bk", [128, 4], f32, kind="ExternalInput")
    io.bvb = nc.dram_tensor("bvb", [128, JC], f32, kind="ExternalInput")
    io.out = nc.dram_tensor("out", [S, D], f16, kind="ExternalOutput")

    with tile.TileContext(nc) as tc:
        with (
            tc.tile_pool(name="big", bufs=1) as big,
            tc.tile_pool(name="work", bufs=3) as work,
            tc.tile_pool(name="xpool", bufs=1) as xp,
            tc.tile_pool(name="xvpool", bufs=4) as xvp,
            tc.tile_pool(name="att", bufs=6) as att,
            tc.tile_pool(name="att2", bufs=2) as att2,
        ):
            sb = _NS()
            sb.qT_sb = big.tile([128, 4, S], f16)           # [p, jt, s]
            sb.kT_sb = big.tile([128, 4, S], f16)
            sb.v_sb = big.tile([128, 16, HC, DK], f16)      # [p, st, h, c]
            sb.ones_sb = big.tile([128, DK], f16)
            sb.wq_sb = big.tile([128, 8, JC], f16)
            sb.wk_sb = big.tile([128, 8, JC], f16)
            sb.wv_sb = big.tile([128, 8, JC], f16)
            sb.bq_sb = big.tile([128, 4], f32)
            sb.bk_sb = big.tile([128, 4], f32)
            sb.bvb_sb = big.tile([128, JC], f32)
            sb.ctxn_sb = big.tile([128, 4, S], f16)         # [p, pair, s]
            sb.wot_sb = big.tile([128, 4, D], f16)

            nc.vector.memset(sb.ones_sb[:], 1.0)

            bias_insts = {}   # (key, jt, sc) -> bias-add instruction
            mul_insts = {}    # (sqb, pair, half) -> [mul instructions]

            def _dep(reader, writer, why):
                # Tile misses DVE-write -> matmul-stationary-read deps when
                # emission is tightly interleaved; add the edge explicitly.
                f = reader.ins if isinstance(reader, bass.BassInstruction) else reader
                t = writer.ins if isinstance(writer, bass.BassInstruction) else writer
                add_dep_helper(f, t, sync=True, reason=why)

            xq = [None, None]
            xk = [None, None]

            def dma_x(x_dram, tag, sc, ts):
                t = xp.tile([128, 8, 1024], f16, tag=f"{tag}{sc}")
                nc.sync.dma_start(
                    t[:],
                    x_dram[:, sc * 1024:(sc + 1) * 1024].rearrange(
                        "(kt p) s -> p kt s", p=128
                    ),
                )
                ts[sc] = t

            def qk_fill(pool, tag, key, x_ts, w_sb, o_sb, b_sb, jt, sc):
                """One [128,1024] projection fill: 16 MMs + bias add."""
                ps = pool.tile([128, 1024], f32, tag=tag,
                               name=f"pj_{key}_{jt}_{sc}")
                for kt in range(8):
                    w = w_sb[:, kt, jt * 128:(jt + 1) * 128]
                    nc.tensor.matmul(ps[:, 0:512], w, x_ts[sc][:, kt, 0:512],
                                     start=(kt == 0), stop=(kt == 7))
                    nc.tensor.matmul(ps[:, 512:1024], w, x_ts[sc][:, kt, 512:1024],
                                     start=(kt == 0), stop=(kt == 7))
                    yield
                bias_insts[(key, jt, sc)] = nc.vector.tensor_scalar_add(
                    o_sb[:, jt, sc * 1024:(sc + 1) * 1024], ps[:], b_sb[:, jt:jt + 1])
                yield

            def run(gen):
                for _ in gen:
                    pass

            # --- upfront: v first; the q/k x/w DMA issues are interleaved
            # into the xv DMA sequence so the Sync queue (which is in-order
            # and paced by the xv slot rotation) doesn't delay them.
            nc.sync.dma_start(sb.wv_sb[:], io.wvt.rearrange("(kt p) j -> p kt j", p=128))
            nc.sync.dma_start(sb.bvb_sb[:], io.bvb[:])
            with (
                tc.tile_pool(name="fat", bufs=2, space="PSUM") as fat,
                tc.tile_pool(name="fatv", bufs=2, space="PSUM") as fatv,
            ):
                for st in range(16):
                    xv = xvp.tile([128, 8, 128], f16, tag="xv")
                    nc.sync.dma_start(
                        xv[:],
                        io.vt[:, st * 128:(st + 1) * 128].rearrange(
                            "(kt p) s -> p kt s", p=128
                        ),
                    )
                    if st == 3:
                        nc.sync.dma_start(
                            sb.wq_sb[:],
                            io.wqt.rearrange("(kt p) j -> p kt j", p=128))
                        nc.sync.dma_start(sb.bq_sb[:], io.bq[:])
                        dma_x(io.qt, "xq", 0, xq)
                    if st == 7:
                        nc.sync.dma_start(
                            sb.wk_sb[:],
                            io.wkt.rearrange("(kt p) j -> p kt j", p=128))
                        nc.sync.dma_start(sb.bk_sb[:], io.bk[:])
                        dma_x(io.kt, "xk", 0, xk)
                    if st == 11:
                        dma_x(io.kt, "xk", 1, xk)
                    ps = fatv.tile([128, JC], f32, tag="projv")
                    for kt in range(8):
                        nc.tensor.matmul(ps[:], xv[:, kt, :], sb.wv_sb[:, kt, :],
                                         start=(kt == 0), stop=(kt == 7))
                    nc.vector.tensor_add(
                        sb.v_sb[:, st, :, :],
                        ps[:].rearrange("p (h c) -> p h c", h=HC),
                        sb.bvb_sb[:].rearrange("p (h c) -> p h c", h=HC),
                    )
                dma_x(io.qt, "xq", 1, xq)
                nc.sync.dma_start(
                    sb.wot_sb[:], io.wot.rearrange("(kt p) j -> p kt j", p=128))
                run(qk_fill(fat, "proj", "q", xq, sb.wq_sb, sb.qT_sb, sb.bq_sb, 0, 0))
                run(qk_fill(fat, "proj", "k", xk, sb.wk_sb, sb.kT_sb, sb.bk_sb, 0, 0))
                run(qk_fill(fat, "proj", "k", xk, sb.wk_sb, sb.kT_sb, sb.bk_sb, 0, 1))

            # --- stage 2/3 with background stage-1 work -------------------
            with (
                tc.tile_pool(name="ps2st", bufs=2, space="PSUM") as pp_st,
                tc.tile_pool(name="ps2cl", bufs=1, space="PSUM") as pp_cl,
                tc.tile_pool(name="pj", bufs=1, space="PSUM") as pp_pj,
            ):
                def stage3_chunk(sq2, tagit):
                    sqb_r, half_r = sq2 // 8, (sq2 % 8) // 4
                    for n in range(2):
                        ps = pp_pj.tile([128, 1024], f32, tag=next(tagit),
                                        name=f"o_{sq2}_{n}")
                        for p in range(4):
                            omm = nc.tensor.matmul(
                                ps[:, 0:512],
                                sb.ctxn_sb[:, p, sq2 * 128:(sq2 + 1) * 128],
                                sb.wot_sb[:, p, n * 512:(n + 1) * 512],
                                start=(p == 0), stop=(p == 3),
                            )
                            if n == 0:
                                for m in mul_insts[(sqb_r, p, half_r)]:
                                    _dep(omm, m, f"out({sq2}) after ctxn")
                            if p % 2:
                                yield
                        ob = work.tile([128, 512], f16, tag="ob")
                        nc.vector.tensor_copy(ob[:], ps[:, 0:512])
                        nc.sync.dma_start(
                            io.out[sq2 * 128:(sq2 + 1) * 128,
                                   n * 512:(n + 1) * 512],
                            ob[:],
                        )
                        yield

                def bg_qk():
                    # ordered by deadline: pair p (group 32p) needs q-sc0,
                    # k-sc0, k-sc1 of jt=p; the q-sc1 fills are only read in
                    # s_q block 1 and trail behind.
                    q_args = ("q", xq, sb.wq_sb, sb.qT_sb, sb.bq_sb)
                    k_args = ("k", xk, sb.wk_sb, sb.kT_sb, sb.bk_sb)
                    for jt in range(1, 4):
                        for (key, x_ts, w_sb, o_sb, b_sb), sc in (
                            (q_args, 0), (k_args, 0), (k_args, 1),
                        ):
                            yield from qk_fill(pp_pj, "pj", key, x_ts, w_sb,
                                               o_sb, b_sb, jt, sc)
                    for jt in range(0, 4):
                        key, x_ts, w_sb, o_sb, b_sb = q_args
                        yield from qk_fill(pp_pj, "pj", key, x_ts, w_sb,
                                           o_sb, b_sb, jt, 1)

                def bg_s3():
                    # output projection for s_q block 0 (runs during block 1)
                    tagit = itertools.cycle(["pj"])
                    for sq2 in range(8):
                        yield from stage3_chunk(sq2, tagit)

                def bg_s3b():
                    # block-1 rows whose ctxn (half 0) is already complete
                    tagit = itertools.cycle(["pj"])
                    for sq2 in range(8, 12):
                        yield from stage3_chunk(sq2, tagit)

                bgs = [bg_qk()]

                def pump():
                    while bgs:
                        try:
                            next(bgs[0])
                            return
                        except StopIteration:
                            bgs.pop(0)

                state = {}

                def emit_cl(g, pt):
                    sqb, pair, half, k = g
                    ctx, lx = state[(sqb, pair, half)]
                    h0, h1 = 2 * pair, 2 * pair + 1
                    st0, sp0 = (k == 0), (k == 15)
                    nc.tensor.matmul(ctx[0:64, :], sb.v_sb[:, k, h0, :],
                                     pt[:, 0:512], start=st0, stop=sp0,
                                     skip_group_check=True)
                    nc.tensor.matmul(ctx[64:128, :], sb.v_sb[:, k, h1, :],
                                     pt[:, 512:1024], start=st0, stop=sp0,
                                     skip_group_check=True)
                    nc.tensor.matmul(lx[0:64, :], sb.ones_sb[:],
                                     pt[:, 0:512], start=st0, stop=sp0,
                                     skip_group_check=True)
                    nc.tensor.matmul(lx[64:128, :], sb.ones_sb[:],
                                     pt[:, 512:1024], start=st0, stop=sp0,
                                     skip_group_check=True)

                def normalize(g):
                    sqb, pair, half, _ = g
                    ctx, lx = state.pop((sqb, pair, half))
                    sq0 = sqb * 1024 + half * 512
                    # evacuate psum first so the ctx/l banks free up for the
                    # next half's matmuls; l is already partition-replicated.
                    cc = att2.tile([128, 512], f32, tag="cc",
                                   name=f"cc_{sqb}_{pair}_{half}")
                    nc.vector.tensor_copy(cc[:], ctx[:])
                    lc = att2.tile([128, 512], f32, tag="lc",
                                   name=f"lc_{sqb}_{pair}_{half}")
                    nc.vector.tensor_copy(lc[:], lx[:])
                    r = att2.tile([128, 512], f32, tag="r",
                                  name=f"r_{sqb}_{pair}_{half}")
                    nc.vector.reciprocal_approx_fast(r[:], lc[:])
                    mul_insts[(sqb, pair, half)] = [
                        nc.vector.tensor_mul(
                            sb.ctxn_sb[0:64, pair, sq0:sq0 + 512],
                            cc[0:64, :], r[0:64, :],
                        ),
                        nc.vector.tensor_mul(
                            sb.ctxn_sb[64:128, pair, sq0:sq0 + 512],
                            cc[64:128, :], r[64:128, :],
                        ),
                    ]

                groups = [(sqb, pair, half, k)
                          for sqb in range(2) for pair in range(4)
                          for half in range(2) for k in range(16)]
                pend = None
                for g in groups:
                    sqb, pair, half, k = g
                    if k == 0:
                        if (sqb, pair, half) == (1, 0, 0):
                            bgs.append(bg_s3())
                        if (sqb, pair, half) == (1, 3, 1):
                            bgs.append(bg_s3b())
                        state[(sqb, pair, half)] = (
                            pp_cl.tile([128, 512], f32, tag="ctx",
                                       name=f"ctx_{sqb}_{pair}_{half}"),
                            pp_cl.tile([128, 512], f32, tag="l",
                                       name=f"l_{sqb}_{pair}_{half}"),
                        )
                    sq0 = sqb * 1024 + half * 512
                    st = pp_st.tile([128, 1024], f32, tag="st")
                    smm = nc.tensor.matmul(
                        st[:, 0:512],
                        sb.kT_sb[0:64, pair, k * 128:(k + 1) * 128],
                        sb.qT_sb[0:64, pair, sq0:sq0 + 512],
                        start=True, stop=True,
                    )
                    if half == 0 and k in (0, 8) and pair + sqb > 0:
                        why = f"scores({sqb},{pair}) after qk bias"
                        if k == 0:
                            _dep(smm, bias_insts[("q", pair, sqb)], why)
                            _dep(smm, bias_insts[("k", pair, 0)], why)
                        else:
                            _dep(smm, bias_insts[("k", pair, 1)], why)
                    nc.tensor.matmul(
                        st[:, 512:1024],
                        sb.kT_sb[64:128, pair, k * 128:(k + 1) * 128],
                        sb.qT_sb[64:128, pair, sq0:sq0 + 512],
                        start=True, stop=True,
                    )
                    pt = att.tile([128, 1024], f16, tag="pt")
                    nc.scalar.activation(pt[:], st[:], EXP, scale=SCALE)
                    if pend is not None:
                        emit_cl(*pend)
                        if pend[0][3] == 15:
                            normalize(pend[0])
                    pump()
                    pend = (g, pt)
                emit_cl(*pend)
                normalize(pend[0])

                # drain any remaining background work
                while bgs:
                    try:
                        next(bgs[0])
                    except StopIteration:
                        bgs.pop(0)

                # final output rows (need the very last ctxn half)
                tagit = itertools.cycle(["pj", "ctx", "l"])
                for sq2 in range(12, 16):
                    for n in range(2):
                        tg = next(tagit)
                        ps = (pp_pj if tg == "pj" else pp_cl).tile(
                            [128, 1024] if tg == "pj" else [128, 512],
                            f32, tag=tg, name=f"o_{sq2}_{n}")
                        for p in range(4):
                            omm = nc.tensor.matmul(
                                ps[:, 0:512],
                                sb.ctxn_sb[:, p, sq2 * 128:(sq2 + 1) * 128],
                                sb.wot_sb[:, p, n * 512:(n + 1) * 512],
                                start=(p == 0), stop=(p == 3),
                            )
                            if n == 0:
                                for m in mul_insts[(sq2 // 8, p, (sq2 % 8) // 4)]:
                                    _dep(omm, m, f"out({sq2}) after ctxn")
                        ob = work.tile([128, 512], f16, tag="ob")
                        nc.vector.tensor_copy(ob[:], ps[:, 0:512])
                        nc.sync.dma_start(
                            io.out[sq2 * 128:(sq2 + 1) * 128,
                                   n * 512:(n + 1) * 512],
                            ob[:],
                        )

    nc.compile()
    return nc


_NC = None


def _get_nc():
    global _NC
    if _NC is None:
        _NC = build_nc()
    return _NC


def make_in_maps(Q, K, V, Wq, bq, Wk, bk, Wv, bv, Wo, bo):
    ash = lambda x: np.ascontiguousarray(np.asarray(x, dtype=np.float32).astype(np.float16))
    asf = lambda x: np.ascontiguousarray(np.asarray(x, dtype=np.float32))
    in_maps = []
    for c in range(N_CORES):
        b = c // 2
        j0 = JC * (c % 2)
        jsl = slice(j0, j0 + JC)
        in_maps.append({
            "qt": ash(np.asarray(Q)[b].T),
            "kt": ash(np.asarray(K)[b].T),
            "vt": ash(np.asarray(V)[b].T),
            "wqt": ash(np.asarray(Wq)[jsl].T),
            "wkt": ash(np.asarray(Wk)[jsl].T),
            "wvt": ash(np.asarray(Wv)[jsl].T),
            "wot": ash(np.asarray(Wo)[:, jsl].T),
            "bq": asf(np.asarray(bq)[jsl].reshape(4, 128).T),
            "bk": asf(np.asarray(bk)[jsl].reshape(4, 128).T),
            "bvb": asf(np.broadcast_to(np.asarray(bv)[jsl], (128, JC))),
        })
    return in_maps


def kernel(Q, K, V, Wq, bq, Wk, bk, Wv, bv, Wo, bo, _trace=False, _trace_kwargs=None):
    nc = _get_nc()
    in_maps = make_in_maps(Q, K, V, Wq, bq, Wk, bk, Wv, bv, Wo, bo)
    res = run_bass_kernel_spmd(
        nc, in_maps, core_ids=list(range(N_CORES)),
        trace=_trace, **(_trace_kwargs or {}),
    )
    parts = [res.results[c]["out"].astype(np.float32) for c in range(N_CORES)]
    bo_np = np.asarray(bo, dtype=np.float32)
    O = np.stack([parts[2 * b] + parts[2 * b + 1] + bo_np for b in range(4)])
    kernel.last_results = res
    return O.astype(np.float32)
